# revision 17
# baseline (speedup 1.0000x reference)
"""Trainium2 Bass kernel for fused attention layer (QKV proj + QKNorm + RoPE +
causal attention + out proj), tensor-parallel across (batch, head-group) on 8
NeuronCores.

Reference semantics (B=2, L=2048, D=2048, H=16, HD=128):
    qkv = x @ w_qkv + b_qkv ; split q,k,v per head
    q,k = rms_norm(q)*q_scale, rms_norm(k)*k_scale   (over head_dim)
    q,k = rope(q), rope(k)                           (rotate-half)
    attn = softmax(mask(q k^T / sqrt(HD)))
    out = (attn @ v) reshaped @ w_out + b_out

Sharding: core c -> batch b=c//4, heads 4g..4g+3 with g=c%4. Each core emits a
partial out-projection [L, D]; the host sums the 4 partials per batch + b_out.
"""

import numpy as np

import concourse.bass as bass
import concourse.bacc as bacc
import concourse.tile as tile
import concourse.mybir as mybir
from concourse.bass_utils import run_bass_kernel_spmd

f32 = mybir.dt.float32
f32r = mybir.dt.float32r
bf16 = mybir.dt.bfloat16
AF = mybir.ActivationFunctionType
ALU = mybir.AluOpType

B = 2
D = 2048
H = 16
HD = 128
KC = D // 128          # 16 contraction chunks of 128
EPS = 1e-6
ROPE_THETA = 10000.0
NEG = -1e30
N_CORES = 8
HEADS_PER_CORE = 4     # 2 pairs of 2


def build_nc(L):
    TQ = L // 512       # 512-wide t chunks (query chunks, proj chunks)
    TK = L // 128       # 128-wide t chunks (key chunks, v chunks, out-proj chunks)
    XC = L // 256       # 256-wide x-stream chunks

    nc = bacc.Bacc(None, target_bir_lowering=False)

    xT_d = nc.dram_tensor("xT", [128, KC, L], f32r, kind="ExternalInput")
    wqk_d = nc.dram_tensor("wqk", [2, 128, KC, 512], f32r, kind="ExternalInput")
    wv_d = nc.dram_tensor("wv", [2, 128, KC, 256], f32r, kind="ExternalInput")
    wout_d = nc.dram_tensor("wout", [128, 4, D], f32r, kind="ExternalInput")
    cos_d = nc.dram_tensor("cosT", [128, L], f32, kind="ExternalInput")
    sin_d = nc.dram_tensor("sinT", [128, L], f32, kind="ExternalInput")  # [-sin; +sin]
    bqk_d = nc.dram_tensor("b_qk", [128, 8], f32, kind="ExternalInput")
    qsks_d = nc.dram_tensor("qs_ks", [128, 2], f32, kind="ExternalInput")
    bv_d = nc.dram_tensor("bv_bc", [128, 2, 256], f32, kind="ExternalInput")
    mask_d = nc.dram_tensor("maskneg", [128, 4, 512], bf16, kind="ExternalInput")  # (p, r, n)
    ones_d = nc.dram_tensor("ones", [128, 1], f32r, kind="ExternalInput")
    out_d = nc.dram_tensor("out_p", [L, D], f32, kind="ExternalOutput")

    inv_sqrt_hd = float(1.0 / np.sqrt(HD))

    # register EPS as a const AP (activation float biases need one)
    _eps_t = nc.alloc_sbuf_tensor(f"const-float32-{EPS}", [128, 1], f32)
    nc.gpsimd.memset(_eps_t.ap(), EPS)
    nc.const_aps.aps[(f32, EPS)] = _eps_t.ap()
    nc.all_engine_barrier()

    from contextlib import ExitStack

    with ExitStack() as ctx:
        tc = ctx.enter_context(tile.TileContext(nc))
        ctx.enter_context(
            nc.allow_low_precision(
                reason="f32r rounding of matmul operands is intentional"
            )
        )
        px = ctx.enter_context(tc.tile_pool(name="px", bufs=2))       # x / wout
        pw = ctx.enter_context(tc.tile_pool(name="pw", bufs=1))       # wqk + wv
        pqk = ctx.enter_context(tc.tile_pool(name="pqk", bufs=4))     # qT/kT
        pv = ctx.enter_context(tc.tile_pool(name="pv", bufs=1))       # v slab
        pat = ctx.enter_context(tc.tile_pool(name="pat", bufs=4))     # attn_outT
        ptab = ctx.enter_context(tc.tile_pool(name="ptab", bufs=1))   # constants
        pscr = ctx.enter_context(tc.tile_pool(name="pscr", bufs=1))   # scratch
        pexp = ctx.enter_context(tc.tile_pool(name="pexp", bufs=2))   # exp tiles
        pbc = ctx.enter_context(tc.tile_pool(name="pbc", bufs=2))     # broadcasts
        psmall = ctx.enter_context(tc.tile_pool(name="psmall", bufs=2))
        postg = ctx.enter_context(tc.tile_pool(name="postg", bufs=2))
        ppsum = ctx.enter_context(tc.tile_pool(name="psum", bufs=4, space="PSUM"))
        ppsum_sm = ctx.enter_context(tc.tile_pool(name="psum_sm", bufs=2, space="PSUM"))
        ppsum_v = ctx.enter_context(tc.tile_pool(name="psum_v", bufs=2, space="PSUM"))
        if True:
            # ---- resident constants ----
            cosT = ptab.tile([128, L], f32, tag="cos")
            sinT = ptab.tile([128, L], f32, tag="sin")
            bqk = ptab.tile([128, 8], f32, tag="bqk")
            qsks = ptab.tile([128, 2], f32, tag="qsks")
            bv = ptab.tile([128, 2, 256], f32, tag="bv")
            masks = ptab.tile([128, 4, 512], bf16, tag="masks")
            ones = ptab.tile([128, 1], f32r, tag="ones")
            nc.sync.dma_start(cosT[:], cos_d[:])
            nc.sync.dma_start(sinT[:], sin_d[:])
            nc.sync.dma_start(bqk[:], bqk_d[:])
            nc.sync.dma_start(qsks[:], qsks_d[:])
            nc.sync.dma_start(bv[:], bv_d[:])
            nc.sync.dma_start(masks[:], mask_d[:])
            nc.sync.dma_start(ones[:], ones_d[:])

            attnT = {}  # (pair, hh) -> [128, L] f32r unnormalized-then-normalized

            for pair in range(2):
                # ---- weights for this pair ----
                wqk = pw.tile([128, KC, 512], f32r, tag="wqk")
                wv = pw.tile([128, KC, 256], f32r, tag="wv")
                nc.sync.dma_start(wqk[:], wqk_d[pair])
                nc.sync.dma_start(wv[:], wv_d[pair])

                vslab = pv.tile([128, TK, 256], f32r, tag="v")
                qk = {}
                for qki in range(2):
                    for hh in range(2):
                        qk[(qki, hh)] = pqk.tile([128, L], f32r, tag="qk", name=f"qk_{qki}_{hh}")

                # ---- projection + norm + rope, per 512-wide t chunk ----
                for c4 in range(TQ):
                    ps_qk = {
                        (qki, hh): ppsum.tile(
                            [128, 512], f32, tag="big", name=f"psqk_{qki}_{hh}"
                        )
                        for qki in range(2)
                        for hh in range(2)
                    }
                    for th in range(2):
                        t0 = c4 * 512 + th * 256
                        xt = px.tile([128, KC, 256], f32r, tag="x")
                        nc.sync.dma_start(xt[:], xT_d[:, :, t0 : t0 + 256])
                        # q/k projections: psum[d_head, t] += w^T x
                        for qki in range(2):
                            for hh in range(2):
                                col = (qki * 2 + hh) * 128
                                for kc in range(KC):
                                    nc.tensor.matmul(
                                        ps_qk[(qki, hh)][:, th * 256 : th * 256 + 256],
                                        wqk[:, kc, col : col + 128],
                                        xt[:, kc, :],
                                        start=(kc == 0),
                                        stop=(kc == KC - 1),
                                    )
                        # v projection: psum[t_sub, d_pair] += x^T w
                        for tsub in range(2):
                            ps_v = ppsum_v.tile([128, 256], f32, tag="vps")
                            for kc in range(KC):
                                nc.tensor.matmul(
                                    ps_v[:],
                                    xt[:, kc, tsub * 128 : tsub * 128 + 128],
                                    wv[:, kc, :],
                                    start=(kc == 0),
                                    stop=(kc == KC - 1),
                                )
                            vi = c4 * 4 + th * 2 + tsub
                            nc.vector.tensor_tensor(
                                vslab[:, vi, :], ps_v[:], bv[:, pair, :], ALU.add
                            )

                    # ---- norm + rope per instance on ps_qk [128, 512] ----
                    for qki in range(2):
                        for hh in range(2):
                            ps = ps_qk[(qki, hh)]
                            bidx = pair * 4 + qki * 2 + hh
                            b_ap = bqk[:, bidx : bidx + 1]
                            tsl = slice(c4 * 512, c4 * 512 + 512)

                            sq = pscr.tile([128, 512], f32r, tag="sq")
                            nc.scalar.activation(sq[:], ps[:], AF.Square, bias=b_ap)
                            ms = ppsum_sm.tile([1, 512], f32, tag="small")
                            nc.tensor.matmul(ms[:], ones[:], sq[:], start=True, stop=True)
                            sms = psmall.tile([1, 512], f32, tag="small")
                            nc.scalar.activation(
                                sms[:], ms[:], AF.Sqrt, bias=EPS, scale=float(1.0 / HD)
                            )
                            rec = psmall.tile([1, 512], f32, tag="small")
                            nc.vector.reciprocal(rec[:], sms[:])
                            rstd = pbc.tile([128, 512], f32, tag="bc")
                            nc.gpsimd.partition_broadcast(rstd[:], rec[:])

                            # q_sb = (ps + bias) * qk_scale (PSUM -> SBUF);
                            # the per-dim scale must precede rope's dim mixing
                            q_sb = pscr.tile([128, 512], f32, tag="q_sb")
                            nc.vector.tensor_scalar(
                                q_sb[:],
                                ps[:],
                                b_ap,
                                qsks[:, qki : qki + 1],
                                ALU.add,
                                ALU.mult,
                            )
                            # rot = rotate-half(q_sb) via SBUF->SBUF DMA
                            rot = pscr.tile([128, 512], f32, tag="rot")
                            nc.sync.dma_start(rot[0:64, :], q_sb[64:128, :])
                            nc.sync.dma_start(rot[64:128, :], q_sb[0:64, :])
                            t1 = pscr.tile([128, 512], f32, tag="t1")
                            nc.vector.tensor_tensor(
                                t1[:], q_sb[:], cosT[:, tsl], ALU.mult
                            )
                            nc.vector.tensor_tensor(
                                rot[:], rot[:], sinT[:, tsl], ALU.mult
                            )
                            nc.vector.tensor_tensor(t1[:], t1[:], rot[:], ALU.add)
                            # qk = t1 * rstd   -> f32r
                            nc.vector.tensor_tensor(
                                qk[(qki, hh)][:, tsl], t1[:], rstd[:], ALU.mult
                            )

                # ---- attention per head ----
                for hh in range(2):
                    qT = qk[(0, hh)]
                    kT = qk[(1, hh)]
                    aT = pat.tile([128, L], f32r, tag="attnT")
                    attnT[(pair, hh)] = aT
                    for j in range(TQ):
                        jsl = slice(j * 512, j * 512 + 512)
                        ncc = 4 * (j + 1)
                        ps_o = ppsum.tile([128, 512], f32, tag="big")
                        ps_sum = ppsum_sm.tile([1, 512], f32, tag="small")
                        for c in range(ncc):
                            ps_s = ppsum.tile([128, 512], f32, tag="big")
                            nc.tensor.matmul(
                                ps_s[:],
                                kT[:, c * 128 : c * 128 + 128],
                                qT[:, jsl],
                                start=True,
                                stop=True,
                            )
                            r = c - 4 * j
                            if r >= 0:
                                nc.vector.tensor_tensor(
                                    ps_s[:], ps_s[:], masks[:, r, :], ALU.add
                                )
                            e = pexp.tile([128, 512], f32r, tag="e")
                            nc.scalar.activation(
                                e[:], ps_s[:], AF.Exp, scale=inv_sqrt_hd
                            )
                            nc.tensor.matmul(
                                ps_sum[:], ones[:], e[:],
                                start=(c == 0), stop=(c == ncc - 1),
                            )
                            nc.tensor.matmul(
                                ps_o[:],
                                vslab[:, c, hh * 128 : hh * 128 + 128],
                                e[:],
                                start=(c == 0),
                                stop=(c == ncc - 1),
                            )
                        reca = psmall.tile([1, 512], f32, tag="small")
                        nc.vector.reciprocal(reca[:], ps_sum[:])
                        rbc = pbc.tile([128, 512], f32, tag="bc")
                        nc.gpsimd.partition_broadcast(rbc[:], reca[:])
                        nc.vector.tensor_tensor(aT[:, jsl], ps_o[:], rbc[:], ALU.mult)

            # ---- out projection ----
            wo = [px.tile([128, 2, D], f32r, tag="x", name=f"wo_{i}") for i in range(2)]
            nc.sync.dma_start(wo[0][:], wout_d[:, 0:2, :])
            nc.sync.dma_start(wo[1][:], wout_d[:, 2:4, :])
            for tc_i in range(TK):
                tsl = slice(tc_i * 128, tc_i * 128 + 128)
                for n in range(D // 512):
                    nsl = slice(n * 512, n * 512 + 512)
                    ps = ppsum.tile([128, 512], f32, tag="big")
                    for hi in range(4):
                        nc.tensor.matmul(
                            ps[:],
                            attnT[(hi // 2, hi % 2)][:, tsl],
                            wo[hi // 2][:, hi % 2, nsl],
                            start=(hi == 0),
                            stop=(hi == 3),
                        )
                    o = postg.tile([128, 512], f32, tag="o")
                    nc.vector.tensor_copy(o[:], ps[:])
                    nc.sync.dma_start(out_d[tsl, nsl], o[:])

    nc.compile()
    return nc


def host_inputs(x, w_qkv, b_qkv, q_scale, k_scale, w_out, L):
    """Build the 8 per-core input maps."""
    x = np.asarray(x, np.float32)
    w_qkv = np.asarray(w_qkv, np.float32)
    b_qkv = np.asarray(b_qkv, np.float32)
    w_out = np.asarray(w_out, np.float32)
    q_scale = np.asarray(q_scale, np.float32)
    k_scale = np.asarray(k_scale, np.float32)

    half = HD // 2
    inv_freq = 1.0 / (ROPE_THETA ** (np.arange(half, dtype=np.float64) / half))
    pos = np.arange(L, dtype=np.float64)
    ang = pos[None, :] * inv_freq[:, None]          # [64, L]
    cos_t = np.cos(ang)
    sin_t = np.sin(ang)
    cosT = np.concatenate([cos_t, cos_t], 0).astype(np.float32)         # [128, L]
    sinT = np.concatenate([-sin_t, sin_t], 0).astype(np.float32)        # [-s; +s]

    # straddle masks r=0..3: keep (0) iff jj >= ii + 128*r else NEG
    ii = np.arange(128)[:, None]
    jj = np.arange(512)[None, :]
    maskneg = np.stack(
        [np.where(jj >= ii + 128 * r, 0.0, NEG) for r in range(4)], axis=1
    ).astype(np.float32)  # [128, 4, 512]
    import ml_dtypes
    maskneg = np.ascontiguousarray(maskneg.astype(ml_dtypes.bfloat16))

    onesv = np.ones((128, 1), np.float32)
    qsks = np.stack([q_scale, k_scale], 1)          # [128, 2]

    in_maps = []
    for c in range(N_CORES):
        b = c // 4
        g = c % 4
        heads = [4 * g + i for i in range(4)]
        xT = np.ascontiguousarray(x[b].T)                       # [D, L]
        xTr = np.ascontiguousarray(
            xT.reshape(KC, 128, L).transpose(1, 0, 2)
        )                                                        # [128, KC, L]
        wqk = np.empty((2, 128, KC, 512), np.float32)
        wv = np.empty((2, 128, KC, 256), np.float32)
        b_qk = np.empty((128, 2, 2, 2), np.float32)  # reshaped to [128, 8] below
        bv_bc = np.empty((128, 2, 256), np.float32)
        for p in range(2):
            hp = heads[2 * p : 2 * p + 2]
            cols = np.concatenate(
                [
                    np.arange(qki * D + h * HD, qki * D + (h + 1) * HD)
                    for qki in range(2)
                    for h in hp
                ]
            )
            wqk[p] = w_qkv[:, cols].reshape(KC, 128, 512).transpose(1, 0, 2)
            vcols = np.concatenate(
                [np.arange(2 * D + h * HD, 2 * D + (h + 1) * HD) for h in hp]
            )
            wv[p] = w_qkv[:, vcols].reshape(KC, 128, 256).transpose(1, 0, 2)
            for qki in range(2):
                for hh in range(2):
                    b_qk[:, p, qki, hh] = b_qkv[
                        qki * D + hp[hh] * HD : qki * D + (hp[hh] + 1) * HD
                    ]
            bv_bc[:, p, :] = np.broadcast_to(b_qkv[vcols], (128, 256))

        wout = (
            w_out[heads[0] * HD : (heads[-1] + 1) * HD]
            .reshape(4, 128, D)
            .transpose(1, 0, 2)
        )
        in_maps.append(
            {
                "xT": np.ascontiguousarray(xTr),
                "wqk": np.ascontiguousarray(wqk),
                "wv": np.ascontiguousarray(wv),
                "wout": np.ascontiguousarray(wout),
                "cosT": cosT,
                "sinT": sinT,
                "b_qk": np.ascontiguousarray(b_qk.reshape(128, 8)),
                "qs_ks": np.ascontiguousarray(qsks),
                "bv_bc": bv_bc,
                "maskneg": maskneg,
                "ones": onesv,
            }
        )
    return in_maps


_NC_CACHE = {}


def _get_nc(L):
    if L not in _NC_CACHE:
        _NC_CACHE[L] = build_nc(L)
    return _NC_CACHE[L]


def run(x, w_qkv, b_qkv, q_scale, k_scale, w_out, b_out, L, **rb_kwargs):
    nc = _get_nc(L)
    in_maps = host_inputs(x, w_qkv, b_qkv, q_scale, k_scale, w_out, L)
    res = run_bass_kernel_spmd(nc, in_maps, list(range(N_CORES)), **rb_kwargs)
    parts = np.stack([r["out_p"] for r in res.results])          # [8, L, D]
    out = np.empty((B, L, D), np.float32)
    for b in range(B):
        out[b] = parts[4 * b : 4 * b + 4].sum(0, dtype=np.float64) + np.asarray(
            b_out, np.float64
        )
    return out, res


def kernel(x, w_qkv, b_qkv, q_scale, k_scale, w_out, b_out, mask):
    out, _ = run(x, w_qkv, b_qkv, q_scale, k_scale, w_out, b_out, L=x.shape[1])
    return out


# revision 20
# speedup vs baseline: 1.1727x; 1.1727x over previous
"""Trainium2 Bass kernel for fused attention layer (QKV proj + QKNorm + RoPE +
causal attention + out proj), tensor-parallel across (batch, head-group) on 8
NeuronCores.

Reference semantics (B=2, L=2048, D=2048, H=16, HD=128):
    qkv = x @ w_qkv + b_qkv ; split q,k,v per head
    q,k = rms_norm(q)*q_scale, rms_norm(k)*k_scale   (over head_dim)
    q,k = rope(q), rope(k)                           (rotate-half)
    attn = softmax(mask(q k^T / sqrt(HD)))
    out = (attn @ v) reshaped @ w_out + b_out

Sharding: core c -> batch b=c//4, heads 4g..4g+3 with g=c%4. Each core emits a
partial out-projection [L, D]; the host sums the 4 partials per batch + b_out.
"""

import numpy as np

import concourse.bass as bass
import concourse.bacc as bacc
import concourse.tile as tile
import concourse.mybir as mybir
from concourse.bass_utils import run_bass_kernel_spmd

f32 = mybir.dt.float32
f32r = mybir.dt.float32r
bf16 = mybir.dt.bfloat16
AF = mybir.ActivationFunctionType
ALU = mybir.AluOpType

B = 2
D = 2048
H = 16
HD = 128
KC = D // 128          # 16 contraction chunks of 128
EPS = 1e-6
ROPE_THETA = 10000.0
NEG = -1e30
N_CORES = 8
HEADS_PER_CORE = 4     # 2 pairs of 2


def build_nc(L):
    TQ = L // 512       # 512-wide t chunks (query chunks, proj chunks)
    TK = L // 128       # 128-wide t chunks (key chunks, v chunks, out-proj chunks)
    XC = L // 256       # 256-wide x-stream chunks

    nc = bacc.Bacc(None, target_bir_lowering=False)

    xT_d = nc.dram_tensor("xT", [128, KC, L], f32r, kind="ExternalInput")
    wqk_d = nc.dram_tensor("wqk", [2, 128, KC, 512], f32r, kind="ExternalInput")
    wv_d = nc.dram_tensor("wv", [2, 128, KC, 256], f32r, kind="ExternalInput")
    wout_d = nc.dram_tensor("wout", [128, 4, D], f32r, kind="ExternalInput")
    cos_d = nc.dram_tensor("cosT", [128, L], f32, kind="ExternalInput")
    sin_d = nc.dram_tensor("sinT", [128, L], f32, kind="ExternalInput")  # [-sin; +sin]
    bqk_d = nc.dram_tensor("b_qk", [128, 8], f32, kind="ExternalInput")
    qsks_d = nc.dram_tensor("qs_ks", [128, 2], f32, kind="ExternalInput")
    bv_d = nc.dram_tensor("bv_bc", [128, 2, 256], f32, kind="ExternalInput")
    mask_d = nc.dram_tensor("maskneg", [128, 4, 512], bf16, kind="ExternalInput")  # (p, r, n)
    ones_d = nc.dram_tensor("ones", [128, 1], f32r, kind="ExternalInput")
    out_d = nc.dram_tensor("out_p", [L, D], f32, kind="ExternalOutput")

    inv_sqrt_hd = float(1.0 / np.sqrt(HD))

    # register EPS as a const AP (activation float biases need one)
    _eps_t = nc.alloc_sbuf_tensor(f"const-float32-{EPS}", [128, 1], f32)
    nc.gpsimd.memset(_eps_t.ap(), EPS)
    nc.const_aps.aps[(f32, EPS)] = _eps_t.ap()
    nc.all_engine_barrier()

    from contextlib import ExitStack

    with ExitStack() as ctx:
        tc = ctx.enter_context(tile.TileContext(nc))
        ctx.enter_context(
            nc.allow_low_precision(
                reason="f32r rounding of matmul operands is intentional"
            )
        )
        px = ctx.enter_context(tc.tile_pool(name="px", bufs=2))       # x / wout
        pw = ctx.enter_context(tc.tile_pool(name="pw", bufs=1))       # wqk + wv
        pqk = ctx.enter_context(tc.tile_pool(name="pqk", bufs=4))     # qT/kT
        pv = ctx.enter_context(tc.tile_pool(name="pv", bufs=1))       # v slab
        pat = ctx.enter_context(tc.tile_pool(name="pat", bufs=4))     # attn_outT
        ptab = ctx.enter_context(tc.tile_pool(name="ptab", bufs=1))   # constants
        pscr = ctx.enter_context(tc.tile_pool(name="pscr", bufs=1))   # scratch
        pexp = ctx.enter_context(tc.tile_pool(name="pexp", bufs=2))   # exp tiles
        pbc = ctx.enter_context(tc.tile_pool(name="pbc", bufs=2))     # broadcasts
        psmall = ctx.enter_context(tc.tile_pool(name="psmall", bufs=2))
        postg = ctx.enter_context(tc.tile_pool(name="postg", bufs=2))
        ppsum = ctx.enter_context(tc.tile_pool(name="psum", bufs=4, space="PSUM"))
        ppsum_sm = ctx.enter_context(tc.tile_pool(name="psum_sm", bufs=2, space="PSUM"))
        ppsum_v = ctx.enter_context(tc.tile_pool(name="psum_v", bufs=2, space="PSUM"))
        if True:
            # ---- resident constants ----
            cosT = ptab.tile([128, L], f32, tag="cos")
            sinT = ptab.tile([128, L], f32, tag="sin")
            bqk = ptab.tile([128, 8], f32, tag="bqk")
            qsks = ptab.tile([128, 2], f32, tag="qsks")
            bv = ptab.tile([128, 2, 256], f32, tag="bv")
            masks = ptab.tile([128, 4, 512], bf16, tag="masks")
            ones = ptab.tile([128, 1], f32r, tag="ones")
            nc.sync.dma_start(cosT[:], cos_d[:])
            nc.sync.dma_start(sinT[:], sin_d[:])
            nc.sync.dma_start(bqk[:], bqk_d[:])
            nc.sync.dma_start(qsks[:], qsks_d[:])
            nc.sync.dma_start(bv[:], bv_d[:])
            nc.sync.dma_start(masks[:], mask_d[:])
            nc.sync.dma_start(ones[:], ones_d[:])

            attnT = {}  # (pair, hh) -> [128, L] f32r unnormalized-then-normalized

            for pair in range(2):
                # ---- weights for this pair ----
                wqk = pw.tile([128, KC, 512], f32r, tag="wqk")
                wv = pw.tile([128, KC, 256], f32r, tag="wv")
                nc.sync.dma_start(wqk[:], wqk_d[pair])
                nc.sync.dma_start(wv[:], wv_d[pair])

                vslab = pv.tile([128, TK, 256], f32r, tag="v")
                qk = {}
                for qki in range(2):
                    for hh in range(2):
                        qk[(qki, hh)] = pqk.tile([128, L], f32r, tag="qk", name=f"qk_{qki}_{hh}")

                # ---- projection + norm + rope, per 512-wide t chunk ----
                for c4 in range(TQ):
                    ps_qk = {
                        (qki, hh): ppsum.tile(
                            [128, 512], f32, tag="big", name=f"psqk_{qki}_{hh}"
                        )
                        for qki in range(2)
                        for hh in range(2)
                    }
                    t0 = c4 * 512
                    # two x tiles, each half the contraction chunks, full 512 t
                    xts = []
                    for kh in range(2):
                        xt = px.tile([128, KC // 2, 512], f32r, tag="x", name=f"xt_{kh}")
                        nc.sync.dma_start(
                            xt[:], xT_d[:, kh * (KC // 2) : (kh + 1) * (KC // 2), t0 : t0 + 512]
                        )
                        xts.append(xt)
                    # q/k projections at N=512: psum[d_head, t] += w^T x
                    for qki in range(2):
                        for hh in range(2):
                            col = (qki * 2 + hh) * 128
                            for kc in range(KC):
                                nc.tensor.matmul(
                                    ps_qk[(qki, hh)][:],
                                    wqk[:, kc, col : col + 128],
                                    xts[kc // (KC // 2)][:, kc % (KC // 2), :],
                                    start=(kc == 0),
                                    stop=(kc == KC - 1),
                                )
                    # v projection: psum[t_sub, d_pair] += x^T w
                    for tsub in range(4):
                        ps_v = ppsum_v.tile([128, 256], f32, tag="vps")
                        for kc in range(KC):
                            nc.tensor.matmul(
                                ps_v[:],
                                xts[kc // (KC // 2)][
                                    :, kc % (KC // 2), tsub * 128 : tsub * 128 + 128
                                ],
                                wv[:, kc, :],
                                start=(kc == 0),
                                stop=(kc == KC - 1),
                            )
                        vi = c4 * 4 + tsub
                        nc.vector.tensor_tensor(
                            vslab[:, vi, :], ps_v[:], bv[:, pair, :], ALU.add
                        )

                    # ---- norm + rope per instance on ps_qk [128, 512] ----
                    for qki in range(2):
                        for hh in range(2):
                            ps = ps_qk[(qki, hh)]
                            bidx = pair * 4 + qki * 2 + hh
                            b_ap = bqk[:, bidx : bidx + 1]
                            tsl = slice(c4 * 512, c4 * 512 + 512)

                            sq = pscr.tile([128, 512], f32r, tag="sq")
                            nc.scalar.activation(sq[:], ps[:], AF.Square, bias=b_ap)
                            ms = ppsum_sm.tile([1, 512], f32, tag="small")
                            nc.tensor.matmul(ms[:], ones[:], sq[:], start=True, stop=True)
                            sms = psmall.tile([1, 512], f32, tag="small")
                            nc.scalar.activation(
                                sms[:], ms[:], AF.Sqrt, bias=EPS, scale=float(1.0 / HD)
                            )
                            rec = psmall.tile([1, 512], f32, tag="small")
                            nc.vector.reciprocal_approx_fast(rec[:], sms[:])
                            rstd = pbc.tile([128, 512], f32, tag="bc")
                            nc.gpsimd.partition_broadcast(rstd[:], rec[:])

                            # q_sb = (ps + bias) * qk_scale (PSUM -> SBUF);
                            # the per-dim scale must precede rope's dim mixing
                            q_sb = pscr.tile([128, 512], f32, tag="q_sb")
                            nc.vector.tensor_scalar(
                                q_sb[:],
                                ps[:],
                                b_ap,
                                qsks[:, qki : qki + 1],
                                ALU.add,
                                ALU.mult,
                            )
                            # rot = rotate-half(q_sb) via SBUF->SBUF DMA
                            rot = pscr.tile([128, 512], f32, tag="rot")
                            nc.sync.dma_start(rot[0:64, :], q_sb[64:128, :])
                            nc.sync.dma_start(rot[64:128, :], q_sb[0:64, :])
                            t1 = pscr.tile([128, 512], f32, tag="t1")
                            nc.vector.tensor_tensor(
                                t1[:], q_sb[:], cosT[:, tsl], ALU.mult
                            )
                            nc.vector.tensor_tensor(
                                rot[:], rot[:], sinT[:, tsl], ALU.mult
                            )
                            nc.vector.tensor_tensor(t1[:], t1[:], rot[:], ALU.add)
                            # qk = t1 * rstd   -> f32r
                            nc.vector.tensor_tensor(
                                qk[(qki, hh)][:, tsl], t1[:], rstd[:], ALU.mult
                            )

                # ---- attention per head ----
                for hh in range(2):
                    qT = qk[(0, hh)]
                    kT = qk[(1, hh)]
                    aT = pat.tile([128, L], f32r, tag="attnT")
                    attnT[(pair, hh)] = aT
                    for j in range(TQ):
                        jsl = slice(j * 512, j * 512 + 512)
                        ncc = 4 * (j + 1)
                        ps_o = ppsum.tile([128, 512], f32, tag="big")
                        ps_sum = ppsum_sm.tile([1, 512], f32, tag="small")
                        for c in range(ncc):
                            ps_s = ppsum.tile([128, 512], f32, tag="big")
                            nc.tensor.matmul(
                                ps_s[:],
                                kT[:, c * 128 : c * 128 + 128],
                                qT[:, jsl],
                                start=True,
                                stop=True,
                            )
                            r = c - 4 * j
                            if r >= 0:
                                nc.vector.tensor_tensor(
                                    ps_s[:], ps_s[:], masks[:, r, :], ALU.add
                                )
                            e = pexp.tile([128, 512], f32r, tag="e")
                            nc.scalar.activation(
                                e[:], ps_s[:], AF.Exp, scale=inv_sqrt_hd
                            )
                            nc.tensor.matmul(
                                ps_sum[:], ones[:], e[:],
                                start=(c == 0), stop=(c == ncc - 1),
                            )
                            nc.tensor.matmul(
                                ps_o[:],
                                vslab[:, c, hh * 128 : hh * 128 + 128],
                                e[:],
                                start=(c == 0),
                                stop=(c == ncc - 1),
                            )
                        reca = psmall.tile([1, 512], f32, tag="small")
                        nc.vector.reciprocal_approx_fast(reca[:], ps_sum[:])
                        rbc = pbc.tile([128, 512], f32, tag="bc")
                        nc.gpsimd.partition_broadcast(rbc[:], reca[:])
                        nc.vector.tensor_tensor(aT[:, jsl], ps_o[:], rbc[:], ALU.mult)

            # ---- out projection ----
            wo = [px.tile([128, 2, D], f32r, tag="x", name=f"wo_{i}") for i in range(2)]
            nc.sync.dma_start(wo[0][:], wout_d[:, 0:2, :])
            nc.sync.dma_start(wo[1][:], wout_d[:, 2:4, :])
            for tc_i in range(TK):
                tsl = slice(tc_i * 128, tc_i * 128 + 128)
                for n in range(D // 512):
                    nsl = slice(n * 512, n * 512 + 512)
                    ps = ppsum.tile([128, 512], f32, tag="big")
                    for hi in range(4):
                        nc.tensor.matmul(
                            ps[:],
                            attnT[(hi // 2, hi % 2)][:, tsl],
                            wo[hi // 2][:, hi % 2, nsl],
                            start=(hi == 0),
                            stop=(hi == 3),
                        )
                    o = postg.tile([128, 512], f32, tag="o")
                    nc.vector.tensor_copy(o[:], ps[:])
                    nc.sync.dma_start(out_d[tsl, nsl], o[:])

    nc.compile()
    return nc


def host_inputs(x, w_qkv, b_qkv, q_scale, k_scale, w_out, L):
    """Build the 8 per-core input maps."""
    x = np.asarray(x, np.float32)
    w_qkv = np.asarray(w_qkv, np.float32)
    b_qkv = np.asarray(b_qkv, np.float32)
    w_out = np.asarray(w_out, np.float32)
    q_scale = np.asarray(q_scale, np.float32)
    k_scale = np.asarray(k_scale, np.float32)

    half = HD // 2
    inv_freq = 1.0 / (ROPE_THETA ** (np.arange(half, dtype=np.float64) / half))
    pos = np.arange(L, dtype=np.float64)
    ang = pos[None, :] * inv_freq[:, None]          # [64, L]
    cos_t = np.cos(ang)
    sin_t = np.sin(ang)
    cosT = np.concatenate([cos_t, cos_t], 0).astype(np.float32)         # [128, L]
    sinT = np.concatenate([-sin_t, sin_t], 0).astype(np.float32)        # [-s; +s]

    # straddle masks r=0..3: keep (0) iff jj >= ii + 128*r else NEG
    ii = np.arange(128)[:, None]
    jj = np.arange(512)[None, :]
    maskneg = np.stack(
        [np.where(jj >= ii + 128 * r, 0.0, NEG) for r in range(4)], axis=1
    ).astype(np.float32)  # [128, 4, 512]
    import ml_dtypes
    maskneg = np.ascontiguousarray(maskneg.astype(ml_dtypes.bfloat16))

    onesv = np.ones((128, 1), np.float32)
    qsks = np.stack([q_scale, k_scale], 1)          # [128, 2]

    in_maps = []
    for c in range(N_CORES):
        b = c // 4
        g = c % 4
        heads = [4 * g + i for i in range(4)]
        xT = np.ascontiguousarray(x[b].T)                       # [D, L]
        xTr = np.ascontiguousarray(
            xT.reshape(KC, 128, L).transpose(1, 0, 2)
        )                                                        # [128, KC, L]
        wqk = np.empty((2, 128, KC, 512), np.float32)
        wv = np.empty((2, 128, KC, 256), np.float32)
        b_qk = np.empty((128, 2, 2, 2), np.float32)  # reshaped to [128, 8] below
        bv_bc = np.empty((128, 2, 256), np.float32)
        for p in range(2):
            hp = heads[2 * p : 2 * p + 2]
            cols = np.concatenate(
                [
                    np.arange(qki * D + h * HD, qki * D + (h + 1) * HD)
                    for qki in range(2)
                    for h in hp
                ]
            )
            wqk[p] = w_qkv[:, cols].reshape(KC, 128, 512).transpose(1, 0, 2)
            vcols = np.concatenate(
                [np.arange(2 * D + h * HD, 2 * D + (h + 1) * HD) for h in hp]
            )
            wv[p] = w_qkv[:, vcols].reshape(KC, 128, 256).transpose(1, 0, 2)
            for qki in range(2):
                for hh in range(2):
                    b_qk[:, p, qki, hh] = b_qkv[
                        qki * D + hp[hh] * HD : qki * D + (hp[hh] + 1) * HD
                    ]
            bv_bc[:, p, :] = np.broadcast_to(b_qkv[vcols], (128, 256))

        wout = (
            w_out[heads[0] * HD : (heads[-1] + 1) * HD]
            .reshape(4, 128, D)
            .transpose(1, 0, 2)
        )
        in_maps.append(
            {
                "xT": np.ascontiguousarray(xTr),
                "wqk": np.ascontiguousarray(wqk),
                "wv": np.ascontiguousarray(wv),
                "wout": np.ascontiguousarray(wout),
                "cosT": cosT,
                "sinT": sinT,
                "b_qk": np.ascontiguousarray(b_qk.reshape(128, 8)),
                "qs_ks": np.ascontiguousarray(qsks),
                "bv_bc": bv_bc,
                "maskneg": maskneg,
                "ones": onesv,
            }
        )
    return in_maps


_NC_CACHE = {}


def _get_nc(L):
    if L not in _NC_CACHE:
        _NC_CACHE[L] = build_nc(L)
    return _NC_CACHE[L]


def run(x, w_qkv, b_qkv, q_scale, k_scale, w_out, b_out, L, **rb_kwargs):
    nc = _get_nc(L)
    in_maps = host_inputs(x, w_qkv, b_qkv, q_scale, k_scale, w_out, L)
    res = run_bass_kernel_spmd(nc, in_maps, list(range(N_CORES)), **rb_kwargs)
    parts = np.stack([r["out_p"] for r in res.results])          # [8, L, D]
    out = np.empty((B, L, D), np.float32)
    for b in range(B):
        out[b] = parts[4 * b : 4 * b + 4].sum(0, dtype=np.float64) + np.asarray(
            b_out, np.float64
        )
    return out, res


def kernel(x, w_qkv, b_qkv, q_scale, k_scale, w_out, b_out, mask):
    out, _ = run(x, w_qkv, b_qkv, q_scale, k_scale, w_out, b_out, L=x.shape[1])
    return out


# revision 22
# speedup vs baseline: 1.2093x; 1.0313x over previous
"""Trainium2 Bass kernel for fused attention layer (QKV proj + QKNorm + RoPE +
causal attention + out proj), tensor-parallel across (batch, head-group) on 8
NeuronCores.

Reference semantics (B=2, L=2048, D=2048, H=16, HD=128):
    qkv = x @ w_qkv + b_qkv ; split q,k,v per head
    q,k = rms_norm(q)*q_scale, rms_norm(k)*k_scale   (over head_dim)
    q,k = rope(q), rope(k)                           (rotate-half)
    attn = softmax(mask(q k^T / sqrt(HD)))
    out = (attn @ v) reshaped @ w_out + b_out

Sharding: core c -> batch b=c//4, heads 4g..4g+3 with g=c%4. Each core emits a
partial out-projection [L, D]; the host sums the 4 partials per batch + b_out.
"""

import numpy as np

import concourse.bass as bass
import concourse.bacc as bacc
import concourse.tile as tile
import concourse.mybir as mybir
from concourse.bass_utils import run_bass_kernel_spmd

f32 = mybir.dt.float32
f32r = mybir.dt.float32r
bf16 = mybir.dt.bfloat16
AF = mybir.ActivationFunctionType
ALU = mybir.AluOpType

B = 2
D = 2048
H = 16
HD = 128
KC = D // 128          # 16 contraction chunks of 128
EPS = 1e-6
ROPE_THETA = 10000.0
NEG = -1e30
N_CORES = 8
HEADS_PER_CORE = 4     # 2 pairs of 2


def build_nc(L):
    TQ = L // 512       # 512-wide t chunks (query chunks, proj chunks)
    TK = L // 128       # 128-wide t chunks (key chunks, v chunks, out-proj chunks)
    XC = L // 256       # 256-wide x-stream chunks

    nc = bacc.Bacc(None, target_bir_lowering=False)

    xT_d = nc.dram_tensor("xT", [128, KC, L], f32r, kind="ExternalInput")
    wqk_d = nc.dram_tensor("wqk", [2, 128, KC, 512], f32r, kind="ExternalInput")
    wv_d = nc.dram_tensor("wv", [2, 128, KC, 256], f32r, kind="ExternalInput")
    wout_d = nc.dram_tensor("wout", [128, 4, D], f32r, kind="ExternalInput")
    cos_d = nc.dram_tensor("cosT", [128, L], f32, kind="ExternalInput")
    sin_d = nc.dram_tensor("sinT", [128, L], f32, kind="ExternalInput")  # [-sin; +sin]
    bqk_d = nc.dram_tensor("b_qk", [128, 8], f32, kind="ExternalInput")
    qsks_d = nc.dram_tensor("qs_ks", [128, 2], f32, kind="ExternalInput")
    bv_d = nc.dram_tensor("bv_bc", [128, 2, 256], f32, kind="ExternalInput")
    mask_d = nc.dram_tensor("maskneg", [128, 896], bf16, kind="ExternalInput")
    ones_d = nc.dram_tensor("ones", [128, 1], f32r, kind="ExternalInput")
    onesb_d = nc.dram_tensor("ones_bf", [128, 1], bf16, kind="ExternalInput")
    out_d = nc.dram_tensor("out_p", [L, D], f32, kind="ExternalOutput")

    inv_sqrt_hd = float(1.0 / np.sqrt(HD))

    # register EPS as a const AP (activation float biases need one)
    _eps_t = nc.alloc_sbuf_tensor(f"const-float32-{EPS}", [128, 1], f32)
    nc.gpsimd.memset(_eps_t.ap(), EPS)
    nc.const_aps.aps[(f32, EPS)] = _eps_t.ap()
    nc.all_engine_barrier()

    from contextlib import ExitStack

    with ExitStack() as ctx:
        tc = ctx.enter_context(tile.TileContext(nc))
        ctx.enter_context(
            nc.allow_low_precision(
                reason="f32r rounding of matmul operands is intentional"
            )
        )
        px = ctx.enter_context(tc.tile_pool(name="px", bufs=5))       # x stream
        pw = ctx.enter_context(tc.tile_pool(name="pw", bufs=1))       # wqk + wv
        pqk = ctx.enter_context(tc.tile_pool(name="pqk", bufs=4))     # qT/kT
        pv = ctx.enter_context(tc.tile_pool(name="pv", bufs=1))       # v slab
        pat = ctx.enter_context(tc.tile_pool(name="pat", bufs=4))     # attn_outT
        ptab = ctx.enter_context(tc.tile_pool(name="ptab", bufs=1))   # constants
        pscr = ctx.enter_context(tc.tile_pool(name="pscr", bufs=1))   # scratch
        pexp = ctx.enter_context(tc.tile_pool(name="pexp", bufs=2))   # exp tiles
        pbc = ctx.enter_context(tc.tile_pool(name="pbc", bufs=2))     # broadcasts
        psmall = ctx.enter_context(tc.tile_pool(name="psmall", bufs=2))
        ppsum = ctx.enter_context(tc.tile_pool(name="psum", bufs=4, space="PSUM"))
        ppsum_sm = ctx.enter_context(tc.tile_pool(name="psum_sm", bufs=2, space="PSUM"))
        ppsum_v = ctx.enter_context(tc.tile_pool(name="psum_v", bufs=2, space="PSUM"))
        if True:
            # ---- resident constants ----
            cosT = ptab.tile([128, L], f32, tag="cos")
            sinT = ptab.tile([128, L], f32, tag="sin")
            bqk = ptab.tile([128, 8], f32, tag="bqk")
            qsks = ptab.tile([128, 2], f32, tag="qsks")
            bv = ptab.tile([128, 2, 256], f32, tag="bv")
            masks = ptab.tile([128, 896], bf16, tag="masks")
            ones = ptab.tile([128, 1], f32r, tag="ones")
            onesb = ptab.tile([128, 1], bf16, tag="onesb")
            nc.sync.dma_start(cosT[:], cos_d[:])
            nc.sync.dma_start(sinT[:], sin_d[:])
            nc.sync.dma_start(bqk[:], bqk_d[:])
            nc.sync.dma_start(qsks[:], qsks_d[:])
            nc.sync.dma_start(bv[:], bv_d[:])
            nc.sync.dma_start(masks[:], mask_d[:])
            nc.sync.dma_start(ones[:], ones_d[:])
            nc.sync.dma_start(onesb[:], onesb_d[:])

            attnT = {}  # (pair, hh) -> [128, L] f32r unnormalized-then-normalized

            for pair in range(2):
                # ---- weights for this pair ----
                wqk = pw.tile([128, KC, 512], f32r, tag="wqk")
                wv = pw.tile([128, KC, 256], f32r, tag="wv")
                nc.sync.dma_start(wqk[:], wqk_d[pair])
                nc.sync.dma_start(wv[:], wv_d[pair])

                vslab = pv.tile([128, TK, 256], f32r, tag="v")
                qk = {}
                for qki in range(2):
                    for hh in range(2):
                        qk[(qki, hh)] = pqk.tile([128, L], f32r, tag="qk", name=f"qk_{qki}_{hh}")

                # ---- projection + norm + rope, per 512-wide t chunk ----
                for c4 in range(TQ):
                    ps_qk = {
                        (qki, hh): ppsum.tile(
                            [128, 512], f32, tag="big", name=f"psqk_{qki}_{hh}"
                        )
                        for qki in range(2)
                        for hh in range(2)
                    }
                    t0 = c4 * 512
                    # four x tiles, each a quarter of the contraction chunks
                    KQ = KC // 4
                    xts = []
                    for kh in range(4):
                        xt = px.tile([128, KQ, 512], f32r, tag="x", name=f"xt_{kh}")
                        nc.sync.dma_start(
                            xt[:], xT_d[:, kh * KQ : (kh + 1) * KQ, t0 : t0 + 512]
                        )
                        xts.append(xt)
                    # q/k projections at N=512: psum[d_head, t] += w^T x
                    for qki in range(2):
                        for hh in range(2):
                            col = (qki * 2 + hh) * 128
                            for kc in range(KC):
                                nc.tensor.matmul(
                                    ps_qk[(qki, hh)][:],
                                    wqk[:, kc, col : col + 128],
                                    xts[kc // KQ][:, kc % KQ, :],
                                    start=(kc == 0),
                                    stop=(kc == KC - 1),
                                )
                    # v projection: psum[t_sub, d_pair] += x^T w
                    for tsub in range(4):
                        ps_v = ppsum_v.tile([128, 256], f32, tag="vps")
                        for kc in range(KC):
                            nc.tensor.matmul(
                                ps_v[:],
                                xts[kc // KQ][
                                    :, kc % KQ, tsub * 128 : tsub * 128 + 128
                                ],
                                wv[:, kc, :],
                                start=(kc == 0),
                                stop=(kc == KC - 1),
                            )
                        vi = c4 * 4 + tsub
                        nc.vector.tensor_tensor(
                            vslab[:, vi, :], ps_v[:], bv[:, pair, :], ALU.add
                        )

                    # ---- norm + rope per instance on ps_qk [128, 512] ----
                    for qki in range(2):
                        for hh in range(2):
                            ps = ps_qk[(qki, hh)]
                            bidx = pair * 4 + qki * 2 + hh
                            b_ap = bqk[:, bidx : bidx + 1]
                            tsl = slice(c4 * 512, c4 * 512 + 512)

                            sq = pscr.tile([128, 512], bf16, tag="sq")
                            nc.scalar.activation(sq[:], ps[:], AF.Square, bias=b_ap)
                            ms = ppsum_sm.tile([1, 512], f32, tag="small")
                            nc.tensor.matmul(ms[:], onesb[:], sq[:], start=True, stop=True)
                            sms = psmall.tile([1, 512], f32, tag="small")
                            nc.scalar.activation(
                                sms[:], ms[:], AF.Sqrt, bias=EPS, scale=float(1.0 / HD)
                            )
                            rec = psmall.tile([1, 512], f32, tag="small")
                            nc.vector.reciprocal_approx_fast(rec[:], sms[:])
                            rstd = pbc.tile([128, 512], f32, tag="bc")
                            nc.gpsimd.partition_broadcast(rstd[:], rec[:])

                            # q_sb = (ps + bias) * qk_scale (PSUM -> SBUF);
                            # the per-dim scale must precede rope's dim mixing
                            q_sb = pscr.tile([128, 512], f32, tag="q_sb")
                            nc.vector.tensor_scalar(
                                q_sb[:],
                                ps[:],
                                b_ap,
                                qsks[:, qki : qki + 1],
                                ALU.add,
                                ALU.mult,
                            )
                            # rot = rotate-half(q_sb) via SBUF->SBUF DMA
                            rot = pscr.tile([128, 512], f32, tag="rot")
                            nc.sync.dma_start(rot[0:64, :], q_sb[64:128, :])
                            nc.sync.dma_start(rot[64:128, :], q_sb[0:64, :])
                            t1 = pscr.tile([128, 512], f32, tag="t1")
                            nc.vector.tensor_tensor(
                                t1[:], q_sb[:], cosT[:, tsl], ALU.mult
                            )
                            nc.vector.tensor_tensor(
                                rot[:], rot[:], sinT[:, tsl], ALU.mult
                            )
                            nc.vector.tensor_tensor(t1[:], t1[:], rot[:], ALU.add)
                            # qk = t1 * rstd   -> f32r
                            nc.vector.tensor_tensor(
                                qk[(qki, hh)][:, tsl], t1[:], rstd[:], ALU.mult
                            )

                # ---- attention per head ----
                for hh in range(2):
                    qT = qk[(0, hh)]
                    kT = qk[(1, hh)]
                    aT = pat.tile([128, L], f32r, tag="attnT")
                    attnT[(pair, hh)] = aT
                    for j in range(TQ):
                        jsl = slice(j * 512, j * 512 + 512)
                        ncc = 4 * (j + 1)
                        ps_o = ppsum.tile([128, 512], f32, tag="big")
                        ps_sum = ppsum_sm.tile([1, 512], f32, tag="small")
                        for c in range(ncc):
                            ps_s = ppsum.tile([128, 512], f32, tag="big")
                            nc.tensor.matmul(
                                ps_s[:],
                                kT[:, c * 128 : c * 128 + 128],
                                qT[:, jsl],
                                start=True,
                                stop=True,
                            )
                            r = c - 4 * j
                            if r >= 0:
                                ms0 = 384 - 128 * r
                                nc.vector.tensor_tensor(
                                    ps_s[:], ps_s[:], masks[:, ms0 : ms0 + 512], ALU.add
                                )
                            e = pexp.tile([128, 512], f32r, tag="e")
                            nc.scalar.activation(
                                e[:], ps_s[:], AF.Exp, scale=inv_sqrt_hd
                            )
                            nc.tensor.matmul(
                                ps_sum[:], ones[:], e[:],
                                start=(c == 0), stop=(c == ncc - 1),
                            )
                            nc.tensor.matmul(
                                ps_o[:],
                                vslab[:, c, hh * 128 : hh * 128 + 128],
                                e[:],
                                start=(c == 0),
                                stop=(c == ncc - 1),
                            )
                        reca = psmall.tile([1, 512], f32, tag="small")
                        nc.vector.reciprocal_approx_fast(reca[:], ps_sum[:])
                        rbc = pbc.tile([128, 512], f32, tag="bc")
                        nc.gpsimd.partition_broadcast(rbc[:], reca[:])
                        nc.vector.tensor_tensor(aT[:, jsl], ps_o[:], rbc[:], ALU.mult)

            # ---- out projection ----
            wo = pw.tile([128, 4, D], f32r, tag="wqk", name="wo")
            nc.sync.dma_start(wo[:], wout_d[:])
            for tc_i in range(TK):
                tsl = slice(tc_i * 128, tc_i * 128 + 128)
                for n in range(D // 512):
                    nsl = slice(n * 512, n * 512 + 512)
                    ps = ppsum.tile([128, 512], f32, tag="big")
                    for hi in range(4):
                        nc.tensor.matmul(
                            ps[:],
                            attnT[(hi // 2, hi % 2)][:, tsl],
                            wo[:, hi, nsl],
                            start=(hi == 0),
                            stop=(hi == 3),
                        )
                    o = pbc.tile([128, 512], f32, tag="bc", name="o_stage")
                    nc.vector.tensor_copy(o[:], ps[:])
                    nc.sync.dma_start(out_d[tsl, nsl], o[:])

    nc.compile()
    return nc


def host_inputs(x, w_qkv, b_qkv, q_scale, k_scale, w_out, L):
    """Build the 8 per-core input maps."""
    x = np.asarray(x, np.float32)
    w_qkv = np.asarray(w_qkv, np.float32)
    b_qkv = np.asarray(b_qkv, np.float32)
    w_out = np.asarray(w_out, np.float32)
    q_scale = np.asarray(q_scale, np.float32)
    k_scale = np.asarray(k_scale, np.float32)

    half = HD // 2
    inv_freq = 1.0 / (ROPE_THETA ** (np.arange(half, dtype=np.float64) / half))
    pos = np.arange(L, dtype=np.float64)
    ang = pos[None, :] * inv_freq[:, None]          # [64, L]
    cos_t = np.cos(ang)
    sin_t = np.sin(ang)
    import ml_dtypes
    cosT = np.concatenate([cos_t, cos_t], 0).astype(np.float32)   # [128, L]
    sinT = np.concatenate([-sin_t, sin_t], 0).astype(np.float32)  # [-s; +s]

    # consolidated straddle mask: M[i, u] = 0 iff u >= i + 384 else NEG;
    # slice [384-128r : 896-128r] gives the r-straddle [128, 512] mask
    ii = np.arange(128)[:, None]
    uu = np.arange(896)[None, :]
    maskneg = np.ascontiguousarray(
        np.where(uu >= ii + 384, 0.0, NEG).astype(ml_dtypes.bfloat16)
    )

    onesv = np.ones((128, 1), np.float32)
    onesb = np.ones((128, 1), ml_dtypes.bfloat16)
    qsks = np.stack([q_scale, k_scale], 1)          # [128, 2]

    in_maps = []
    for c in range(N_CORES):
        b = c // 4
        g = c % 4
        heads = [4 * g + i for i in range(4)]
        xT = np.ascontiguousarray(x[b].T)                       # [D, L]
        xTr = np.ascontiguousarray(
            xT.reshape(KC, 128, L).transpose(1, 0, 2)
        )                                                        # [128, KC, L]
        wqk = np.empty((2, 128, KC, 512), np.float32)
        wv = np.empty((2, 128, KC, 256), np.float32)
        b_qk = np.empty((128, 2, 2, 2), np.float32)  # reshaped to [128, 8] below
        bv_bc = np.empty((128, 2, 256), np.float32)
        for p in range(2):
            hp = heads[2 * p : 2 * p + 2]
            cols = np.concatenate(
                [
                    np.arange(qki * D + h * HD, qki * D + (h + 1) * HD)
                    for qki in range(2)
                    for h in hp
                ]
            )
            wqk[p] = w_qkv[:, cols].reshape(KC, 128, 512).transpose(1, 0, 2)
            vcols = np.concatenate(
                [np.arange(2 * D + h * HD, 2 * D + (h + 1) * HD) for h in hp]
            )
            wv[p] = w_qkv[:, vcols].reshape(KC, 128, 256).transpose(1, 0, 2)
            for qki in range(2):
                for hh in range(2):
                    b_qk[:, p, qki, hh] = b_qkv[
                        qki * D + hp[hh] * HD : qki * D + (hp[hh] + 1) * HD
                    ]
            bv_bc[:, p, :] = np.broadcast_to(b_qkv[vcols], (128, 256))

        wout = (
            w_out[heads[0] * HD : (heads[-1] + 1) * HD]
            .reshape(4, 128, D)
            .transpose(1, 0, 2)
        )
        in_maps.append(
            {
                "xT": np.ascontiguousarray(xTr),
                "wqk": np.ascontiguousarray(wqk),
                "wv": np.ascontiguousarray(wv),
                "wout": np.ascontiguousarray(wout),
                "cosT": cosT,
                "sinT": sinT,
                "b_qk": np.ascontiguousarray(b_qk.reshape(128, 8)),
                "qs_ks": np.ascontiguousarray(qsks),
                "bv_bc": bv_bc,
                "maskneg": maskneg,
                "ones": onesv,
                "ones_bf": onesb,
            }
        )
    return in_maps


_NC_CACHE = {}


def _get_nc(L):
    if L not in _NC_CACHE:
        _NC_CACHE[L] = build_nc(L)
    return _NC_CACHE[L]


def run(x, w_qkv, b_qkv, q_scale, k_scale, w_out, b_out, L, **rb_kwargs):
    nc = _get_nc(L)
    in_maps = host_inputs(x, w_qkv, b_qkv, q_scale, k_scale, w_out, L)
    res = run_bass_kernel_spmd(nc, in_maps, list(range(N_CORES)), **rb_kwargs)
    parts = np.stack([r["out_p"] for r in res.results])          # [8, L, D]
    out = np.empty((B, L, D), np.float32)
    for b in range(B):
        out[b] = parts[4 * b : 4 * b + 4].sum(0, dtype=np.float64) + np.asarray(
            b_out, np.float64
        )
    return out, res


def kernel(x, w_qkv, b_qkv, q_scale, k_scale, w_out, b_out, mask):
    out, _ = run(x, w_qkv, b_qkv, q_scale, k_scale, w_out, b_out, L=x.shape[1])
    return out
